# revision 14
# baseline (speedup 1.0000x reference)
"""CovaBlock kernel for 8 trn2 NeuronCores.

reference computation:
  cova[w] = covariance of support class w over its 8*32*32 = 8192 pixels  [16,128,128]
  qn[b]   = x1[b] with each channel row L2-normalized over the 1024 pixels
  sim[b, w, i] = qn[b,:,i]^T @ cova[w] @ qn[b,:,i]   -> [256, 16*1024]

Plan:
  launch 1: shard the 16 classes over 8 cores (2 each). x2 arrives
    pre-transposed to [128 pix, 64 chunk, 128 ch] bf16 (host does the
    transpose), so the device only runs 64 accumulate matmuls per class:
    E = sum_j X_j X_j^T. The -N m m^T correction happens on the host.
  host:     cova = E/(N-1) - N/(N-1) m m^T, cholesky for the ACT classes.
  launch 2: shard the 256 queries over 8 cores (32 each), processed in
    groups of 8 so each class's stationary is loaded once per 8 queries.
    For ACT-path classes: U = L_w^T qn (PE bf16), V = U^2 (ACT square)
    For DVE-path classes: U = C_w qn  (PE bf16), V = U * qn (DVE mult)
    sim rows = ones^T V (PE ones-matmuls, 4 classes packed per psum tile
    via col tile_position), staged to SBUF (ACT/DVE alternate), DMA out.
    The channel-norm square+accumulate runs on the Pool engine.
"""

import os

import numpy as np

import concourse.bass as bass
import concourse.tile as tile
from concourse import bacc, mybir
from concourse.bass_utils import run_bass_kernel_spmd

F32 = mybir.dt.float32
BF16 = mybir.dt.bfloat16

N_CORES = 8
B, C, HW = 256, 128, 1024          # x1: [B, C, 32, 32]
W, S = 16, 8                       # x2: [W, S, C, 32, 32]
NS = S * HW                        # samples per class = 8192
NCHUNK = NS // 128                 # 64 pixel-chunks per class
BS = B // N_CORES                  # 32 queries per core
WS = W // N_CORES                  # 2 classes per core
GB = 8                             # query group size (stationary reuse)

# classes 0..ACT_W-1 take the cholesky/square path (ACT engine),
# classes ACT_W..15 take the direct/multiply path (DVE engine)
ACT_W = 8

_CACHE = {}


def _build_cova_nc():
    """Per-core: x2t pair [2, 128, 64, 128] bf16 -> E pair [2, 128, 128] f32
    (raw sum_j X_j X_j^T; scaling and mean correction happen on the host)."""
    nc = bacc.Bacc("TRN2", target_bir_lowering=False, debug=False,
                   num_devices=N_CORES)
    x2t = nc.dram_tensor("x2t", [WS, 128, NCHUNK, C], BF16,
                         kind="ExternalInput").ap()
    eout = nc.dram_tensor("e_pair", [WS, C, C], F32, kind="ExternalOutput").ap()

    with tile.TileContext(nc) as tc:
        with (
            tc.tile_pool(name="xt", bufs=2) as xtp,
            tc.tile_pool(name="eo", bufs=2) as eop,
            tc.tile_pool(name="pe", bufs=2, space="PSUM") as pep,
        ):
            for w in range(WS):
                xt = xtp.tile([128, NCHUNK, C], BF16, tag="xt")
                nc.default_dma_engine.dma_start(out=xt, in_=x2t[w])
                e_ps = pep.tile([C, C], F32, tag="E")
                for j in range(NCHUNK):
                    xj = xt[:, j, :]
                    nc.tensor.matmul(e_ps, xj, xj,
                                     start=(j == 0), stop=(j == NCHUNK - 1))
                esb = eop.tile([C, C], F32, tag="esb")
                if w % 2 == 0:
                    nc.scalar.copy(esb, e_ps)
                else:
                    nc.vector.tensor_copy(esb, e_ps)
                nc.default_dma_engine.dma_start(out=eout[w], in_=esb)

    nc.compile()
    return nc


def _build_sim_nc():
    """Per-core: x1 shard [32, 128, 1024] bf16 + mats [16, 128, 128] ->
    sim shard [32, 16 * 1024] f32.

    mats[w] = chol(cova[w]) for w < ACT_W else cova[w], prepacked on host.
    """
    nc = bacc.Bacc("TRN2", target_bir_lowering=False, debug=False,
                   num_devices=N_CORES)
    x1s = nc.dram_tensor("x1s", [BS, C, HW], BF16, kind="ExternalInput").ap()
    mats = nc.dram_tensor("mats", [W, C, C], F32, kind="ExternalInput").ap()
    out = nc.dram_tensor("sim", [BS, W, HW], F32, kind="ExternalOutput").ap()

    HC = 512  # matmul moving-dim chunk

    with tile.TileContext(nc) as tc:
        with (
            tc.tile_pool(name="consts", bufs=1) as consts,
            tc.tile_pool(name="mats", bufs=1) as matp,
            tc.tile_pool(name="xb", bufs=GB + 2) as xbp,
            tc.tile_pool(name="sq", bufs=2) as sqp,
            tc.tile_pool(name="qn", bufs=GB + 1) as qnp,
            tc.tile_pool(name="vv", bufs=4 * GB + 2) as vvp,
            tc.tile_pool(name="st", bufs=4) as stp,
            tc.tile_pool(name="small", bufs=2 * GB + 2) as small,
            tc.tile_pool(name="pu", bufs=3, space="PSUM") as pu,
            tc.tile_pool(name="pr", bufs=2, space="PSUM") as pr,
        ):
            # M=32 all-ones stationary: every column of a col-group strip
            # computes the same k-sum; row j*32 of the strip carries class j's
            # sums.
            ones_r = consts.tile([C, 32], BF16)
            nc.vector.memset(ones_r, 1.0)

            # stationary matrices in bf16 (1 cycle/row at any p-state)
            mat_raw = matp.tile([C, W, C], F32)
            for w in range(W):
                nc.default_dma_engine.dma_start(out=mat_raw[:, w, :], in_=mats[w])
            mat = matp.tile([C, W, C], BF16)
            nc.vector.tensor_copy(mat, mat_raw)

            for bg in range(BS // GB):
                bs = range(bg * GB, (bg + 1) * GB)
                qns = {}
                for b in bs:
                    xb = xbp.tile([C, HW], BF16, tag="xb")
                    nc.default_dma_engine.dma_start(out=xb, in_=x1s[b])
                    # channel norms: n2 = sum_i x^2
                    sq = sqp.tile([C, HW], BF16, tag="sq")
                    n2 = small.tile([C, 1], F32, tag="n2")
                    nc.scalar.activation(sq, xb,
                                         mybir.ActivationFunctionType.Square,
                                         accum_out=n2)
                    nrm = small.tile([C, 1], F32, tag="nrm")
                    nc.scalar.sqrt(nrm, n2)
                    rinv = small.tile([C, 1], F32, tag="rinv")
                    nc.vector.reciprocal(rinv, nrm)
                    # all-bf16 SBUF operands -> DVE 4x mode
                    qn = qnp.tile([C, HW], BF16, tag="qn")
                    nc.vector.tensor_scalar_mul(qn, xb, rinv)
                    qns[b] = qn

                for g in range(4):
                    # projections class-major: one stationary load serves all
                    # GB queries (2 matmuls each); V evacuation trails on
                    # ACT/DVE.
                    vs = {}
                    for j in range(4):
                        w = 4 * g + j
                        lw = mat[:, w, :]
                        for b in bs:
                            u_ps = pu.tile([C, HW], F32, tag="u")
                            for h in range(2):
                                cols = slice(h * HC, (h + 1) * HC)
                                nc.tensor.matmul(u_ps[:, cols], lw,
                                                 qns[b][:, cols],
                                                 start=True, stop=True)
                            v = vvp.tile([C, HW], BF16, tag="v")
                            if w < ACT_W:
                                nc.scalar.square(v, u_ps)
                            else:
                                nc.vector.tensor_mul(v, u_ps, qns[b])
                            vs[(b, j)] = v
                    # reduces per query: j-major, h pairs share the ones
                    # stationary position so walrus elides the reload.
                    for b in bs:
                        red0 = pr.tile([C, HC], F32, tag="red")
                        red1 = pr.tile([C, HC], F32, tag="red")
                        reds = [red0, red1]
                        for j in range(4):
                            for h in range(2):
                                cols = slice(h * HC, (h + 1) * HC)
                                nc.tensor.matmul(
                                    reds[h][32 * j : 32 * j + 32, :],
                                    ones_r, vs[(b, j)][:, cols],
                                    start=True, stop=True,
                                    tile_position=(0, 32 * j))
                        for h in range(2):
                            stage = stp.tile([C, HC], F32, tag="stage")
                            if (b + h) % 2 == 0:
                                nc.scalar.copy(stage, reds[h])
                            else:
                                nc.vector.tensor_copy(stage, reds[h])
                            srows = stage.rearrange(
                                "(j p) n -> j p n", p=32)[:, 0, :]
                            nc.default_dma_engine.dma_start(
                                out=out[b, 4 * g : 4 * g + 4,
                                        h * HC : (h + 1) * HC],
                                in_=srows)

    nc.compile()
    return nc


def kernel(x1: np.ndarray, x2: np.ndarray) -> np.ndarray:
    import ml_dtypes
    x1 = np.asarray(x1, dtype=np.float32).reshape(B, C, HW)
    x2 = np.asarray(x2, dtype=np.float32).reshape(W, S, C, HW)
    core_ids = list(range(N_CORES))

    profile = bool(os.environ.get("COVA_PROFILE"))
    kw1, kw2 = {}, {}
    if profile:
        import shutil
        for d in ("/tmp/cova_prof1", "/tmp/cova_prof2"):
            shutil.rmtree(d, ignore_errors=True)
            os.makedirs(d)
        kw1 = dict(trace=True, tmpdir="/tmp/cova_prof1")
        kw2 = dict(trace=True, tmpdir="/tmp/cova_prof2")

    # host: transpose supports to [W, 128 pix, 64 chunk, 128 ch] bf16
    x2t = np.ascontiguousarray(
        x2.reshape(W, S, C, 8, 128).transpose(0, 4, 1, 3, 2)
        .reshape(W, 128, NCHUNK, C).astype(ml_dtypes.bfloat16))

    if "cova" not in _CACHE:
        _CACHE["cova"] = _build_cova_nc()
    cova_in = [{"x2t": np.ascontiguousarray(x2t[WS * k : WS * (k + 1)])}
               for k in range(N_CORES)]
    res1 = run_bass_kernel_spmd(_CACHE["cova"], cova_in, core_ids, **kw1)
    E = np.concatenate([res1.results[k]["e_pair"] for k in range(N_CORES)], 0)

    # host: cova = E/(N-1) - N/(N-1) m m^T  (mean from the f32 data)
    m = x2.transpose(0, 2, 1, 3).reshape(W, C, NS).mean(axis=2)  # [W, C]
    cova = E.astype(np.float64) / (NS - 1) - (
        NS / (NS - 1.0)) * np.einsum("wc,wd->wcd", m, m, dtype=np.float64)
    chol = np.linalg.cholesky(cova).astype(np.float32)
    mats = np.ascontiguousarray(
        np.concatenate([chol[:ACT_W], cova[ACT_W:].astype(np.float32)], 0))

    x1b = np.ascontiguousarray(x1.astype(ml_dtypes.bfloat16))
    if "sim" not in _CACHE:
        _CACHE["sim"] = _build_sim_nc()
    sim_in = [{"x1s": np.ascontiguousarray(x1b[BS * k : BS * (k + 1)]),
               "mats": mats} for k in range(N_CORES)]
    res2 = run_bass_kernel_spmd(_CACHE["sim"], sim_in, core_ids, **kw2)
    if profile:
        _CACHE["exec_ns"] = (res1.exec_time_ns, res2.exec_time_ns)
    sim = np.concatenate([res2.results[k]["sim"] for k in range(N_CORES)], 0)
    return sim.reshape(B, W * HW)


# revision 15
# speedup vs baseline: 1.1371x; 1.1371x over previous
"""CovaBlock kernel for 8 trn2 NeuronCores.

reference computation:
  cova[w] = covariance of support class w over its 8*32*32 = 8192 pixels  [16,128,128]
  qn[b]   = x1[b] with each channel row L2-normalized over the 1024 pixels
  sim[b, w, i] = qn[b,:,i]^T @ cova[w] @ qn[b,:,i]   -> [256, 16*1024]

Plan:
  launch 1: shard the 16 classes over 8 cores (2 each). x2 arrives
    pre-transposed to [128 pix, 64 chunk, 128 ch] bf16 (host does the
    transpose), so the device only runs 64 accumulate matmuls per class:
    E = sum_j X_j X_j^T. The -N m m^T correction happens on the host.
  host:     cova = E/(N-1) - N/(N-1) m m^T, cholesky for the ACT classes.
  launch 2: shard the 256 queries over 8 cores (32 each), processed in
    groups of 8 so each class's stationary is loaded once per 8 queries.
    For ACT-path classes: U = L_w^T qn (PE bf16), V = U^2 (ACT square)
    For DVE-path classes: U = C_w qn  (PE bf16), V = U * qn (DVE mult)
    sim rows = ones^T V (PE ones-matmuls, 4 classes packed per psum tile
    via col tile_position), staged to SBUF (ACT/DVE alternate), DMA out.
    The channel-norm square+accumulate runs on the Pool engine.
"""

import os

import numpy as np

import concourse.bass as bass
import concourse.tile as tile
from concourse import bacc, mybir
from concourse.bass_utils import run_bass_kernel_spmd

F32 = mybir.dt.float32
BF16 = mybir.dt.bfloat16

N_CORES = 8
B, C, HW = 256, 128, 1024          # x1: [B, C, 32, 32]
W, S = 16, 8                       # x2: [W, S, C, 32, 32]
NS = S * HW                        # samples per class = 8192
NCHUNK = NS // 128                 # 64 pixel-chunks per class
BS = B // N_CORES                  # 32 queries per core
WS = W // N_CORES                  # 2 classes per core
GB = 8                             # query group size (stationary reuse)

# even classes take the cholesky/square path (ACT engine), odd classes the
# direct/multiply path (DVE engine) — parity split so every quad of classes
# keeps both engines busy concurrently
def _act_path(w):
    return w % 2 == 0

_CACHE = {}


def _build_cova_nc():
    """Per-core: x2t pair [2, 128, 64, 128] bf16 -> E pair [2, 128, 128] f32
    (raw sum_j X_j X_j^T; scaling and mean correction happen on the host)."""
    nc = bacc.Bacc("TRN2", target_bir_lowering=False, debug=False,
                   num_devices=N_CORES)
    x2t = nc.dram_tensor("x2t", [WS, 128, NCHUNK, C], BF16,
                         kind="ExternalInput").ap()
    eout = nc.dram_tensor("e_pair", [WS, C, C], F32, kind="ExternalOutput").ap()

    with tile.TileContext(nc) as tc:
        with (
            tc.tile_pool(name="xt", bufs=2) as xtp,
            tc.tile_pool(name="eo", bufs=2) as eop,
            tc.tile_pool(name="pe", bufs=2, space="PSUM") as pep,
        ):
            for w in range(WS):
                xt = xtp.tile([128, NCHUNK, C], BF16, tag="xt")
                nc.default_dma_engine.dma_start(out=xt, in_=x2t[w])
                e_ps = pep.tile([C, C], F32, tag="E")
                for j in range(NCHUNK):
                    xj = xt[:, j, :]
                    nc.tensor.matmul(e_ps, xj, xj,
                                     start=(j == 0), stop=(j == NCHUNK - 1))
                esb = eop.tile([C, C], F32, tag="esb")
                if w % 2 == 0:
                    nc.scalar.copy(esb, e_ps)
                else:
                    nc.vector.tensor_copy(esb, e_ps)
                nc.default_dma_engine.dma_start(out=eout[w], in_=esb)

    nc.compile()
    return nc


def _build_sim_nc():
    """Per-core: x1 shard [32, 128, 1024] bf16 + mats [16, 128, 128] ->
    sim shard [32, 16 * 1024] f32.

    mats[w] = chol(cova[w]) for even w else cova[w], prepacked on host.
    """
    nc = bacc.Bacc("TRN2", target_bir_lowering=False, debug=False,
                   num_devices=N_CORES)
    x1s = nc.dram_tensor("x1s", [BS, C, HW], BF16, kind="ExternalInput").ap()
    mats = nc.dram_tensor("mats", [W, C, C], F32, kind="ExternalInput").ap()
    out = nc.dram_tensor("sim", [BS, W, HW], F32, kind="ExternalOutput").ap()

    HC = 512  # matmul moving-dim chunk

    with tile.TileContext(nc) as tc:
        with (
            tc.tile_pool(name="consts", bufs=1) as consts,
            tc.tile_pool(name="mats", bufs=1) as matp,
            tc.tile_pool(name="xb", bufs=GB + 2) as xbp,
            tc.tile_pool(name="sq", bufs=2) as sqp,
            tc.tile_pool(name="qn", bufs=GB + 1) as qnp,
            tc.tile_pool(name="vv", bufs=4 * GB + 2) as vvp,
            tc.tile_pool(name="st", bufs=4) as stp,
            tc.tile_pool(name="small", bufs=2 * GB + 2) as small,
            tc.tile_pool(name="pu", bufs=3, space="PSUM") as pu,
            tc.tile_pool(name="pr", bufs=2, space="PSUM") as pr,
        ):
            # M=32 all-ones stationary: every column of a col-group strip
            # computes the same k-sum; row j*32 of the strip carries class j's
            # sums.
            ones_r = consts.tile([C, 32], BF16)
            nc.vector.memset(ones_r, 1.0)

            # stationary matrices in bf16 (1 cycle/row at any p-state)
            mat_raw = matp.tile([C, W, C], F32)
            for w in range(W):
                nc.default_dma_engine.dma_start(out=mat_raw[:, w, :], in_=mats[w])
            mat = matp.tile([C, W, C], BF16)
            nc.vector.tensor_copy(mat, mat_raw)

            for bg in range(BS // GB):
                bs = range(bg * GB, (bg + 1) * GB)
                qns = {}
                for b in bs:
                    xb = xbp.tile([C, HW], BF16, tag="xb")
                    nc.default_dma_engine.dma_start(out=xb, in_=x1s[b])
                    # channel norms: n2 = sum_i x^2
                    sq = sqp.tile([C, HW], BF16, tag="sq")
                    n2 = small.tile([C, 1], F32, tag="n2")
                    nc.scalar.activation(sq, xb,
                                         mybir.ActivationFunctionType.Square,
                                         accum_out=n2)
                    nrm = small.tile([C, 1], F32, tag="nrm")
                    nc.scalar.sqrt(nrm, n2)
                    rinv = small.tile([C, 1], F32, tag="rinv")
                    nc.vector.reciprocal(rinv, nrm)
                    # all-bf16 SBUF operands -> DVE 4x mode
                    qn = qnp.tile([C, HW], BF16, tag="qn")
                    nc.vector.tensor_scalar_mul(qn, xb, rinv)
                    qns[b] = qn

                for g in range(4):
                    # projections class-major: one stationary load serves all
                    # GB queries (2 matmuls each); V evacuation trails on
                    # ACT/DVE.
                    vs = {}
                    for j in range(4):
                        w = 4 * g + j
                        lw = mat[:, w, :]
                        for b in bs:
                            u_ps = pu.tile([C, HW], F32, tag="u")
                            for h in range(2):
                                cols = slice(h * HC, (h + 1) * HC)
                                nc.tensor.matmul(u_ps[:, cols], lw,
                                                 qns[b][:, cols],
                                                 start=True, stop=True)
                            v = vvp.tile([C, HW], BF16, tag="v")
                            if _act_path(w):
                                nc.scalar.square(v, u_ps)
                            else:
                                nc.vector.tensor_mul(v, u_ps, qns[b])
                            vs[(b, j)] = v
                    # reduces per query: j-major, h pairs share the ones
                    # stationary position so walrus elides the reload.
                    for b in bs:
                        red0 = pr.tile([C, HC], F32, tag="red")
                        red1 = pr.tile([C, HC], F32, tag="red")
                        reds = [red0, red1]
                        for j in range(4):
                            for h in range(2):
                                cols = slice(h * HC, (h + 1) * HC)
                                nc.tensor.matmul(
                                    reds[h][32 * j : 32 * j + 32, :],
                                    ones_r, vs[(b, j)][:, cols],
                                    start=True, stop=True,
                                    tile_position=(0, 32 * j))
                        for h in range(2):
                            stage = stp.tile([C, HC], F32, tag="stage")
                            if (b + h) % 2 == 0:
                                nc.scalar.copy(stage, reds[h])
                            else:
                                nc.vector.tensor_copy(stage, reds[h])
                            srows = stage.rearrange(
                                "(j p) n -> j p n", p=32)[:, 0, :]
                            nc.default_dma_engine.dma_start(
                                out=out[b, 4 * g : 4 * g + 4,
                                        h * HC : (h + 1) * HC],
                                in_=srows)

    nc.compile()
    return nc


def kernel(x1: np.ndarray, x2: np.ndarray) -> np.ndarray:
    import ml_dtypes
    x1 = np.asarray(x1, dtype=np.float32).reshape(B, C, HW)
    x2 = np.asarray(x2, dtype=np.float32).reshape(W, S, C, HW)
    core_ids = list(range(N_CORES))

    profile = bool(os.environ.get("COVA_PROFILE"))
    kw1, kw2 = {}, {}
    if profile:
        import shutil
        for d in ("/tmp/cova_prof1", "/tmp/cova_prof2"):
            shutil.rmtree(d, ignore_errors=True)
            os.makedirs(d)
        kw1 = dict(trace=True, tmpdir="/tmp/cova_prof1")
        kw2 = dict(trace=True, tmpdir="/tmp/cova_prof2")

    # host: transpose supports to [W, 128 pix, 64 chunk, 128 ch] bf16
    x2t = np.ascontiguousarray(
        x2.reshape(W, S, C, 8, 128).transpose(0, 4, 1, 3, 2)
        .reshape(W, 128, NCHUNK, C).astype(ml_dtypes.bfloat16))

    if "cova" not in _CACHE:
        _CACHE["cova"] = _build_cova_nc()
    cova_in = [{"x2t": np.ascontiguousarray(x2t[WS * k : WS * (k + 1)])}
               for k in range(N_CORES)]
    res1 = run_bass_kernel_spmd(_CACHE["cova"], cova_in, core_ids, **kw1)
    E = np.concatenate([res1.results[k]["e_pair"] for k in range(N_CORES)], 0)

    # host: cova = E/(N-1) - N/(N-1) m m^T  (mean from the f32 data)
    m = x2.transpose(0, 2, 1, 3).reshape(W, C, NS).mean(axis=2)  # [W, C]
    cova = E.astype(np.float64) / (NS - 1) - (
        NS / (NS - 1.0)) * np.einsum("wc,wd->wcd", m, m, dtype=np.float64)
    chol = np.linalg.cholesky(cova).astype(np.float32)
    mats = np.where((np.arange(W) % 2 == 0)[:, None, None],
                    chol, cova.astype(np.float32))
    mats = np.ascontiguousarray(mats.astype(np.float32))

    x1b = np.ascontiguousarray(x1.astype(ml_dtypes.bfloat16))
    if "sim" not in _CACHE:
        _CACHE["sim"] = _build_sim_nc()
    sim_in = [{"x1s": np.ascontiguousarray(x1b[BS * k : BS * (k + 1)]),
               "mats": mats} for k in range(N_CORES)]
    res2 = run_bass_kernel_spmd(_CACHE["sim"], sim_in, core_ids, **kw2)
    if profile:
        _CACHE["exec_ns"] = (res1.exec_time_ns, res2.exec_time_ns)
    sim = np.concatenate([res2.results[k]["sim"] for k in range(N_CORES)], 0)
    return sim.reshape(B, W * HW)


# revision 17
# speedup vs baseline: 1.3109x; 1.1528x over previous
"""CovaBlock kernel for 8 trn2 NeuronCores.

reference computation:
  cova[w] = covariance of support class w over its 8*32*32 = 8192 pixels  [16,128,128]
  qn[b]   = x1[b] with each channel row L2-normalized over the 1024 pixels
  sim[b, w, i] = qn[b,:,i]^T @ cova[w] @ qn[b,:,i]   -> [256, 16*1024]

Plan:
  launch 1: shard the 16 classes over 8 cores (2 each). x2 arrives
    pre-transposed to [128 pix, 64 chunk, 128 ch] bf16 (host does the
    transpose), so the device only runs 64 accumulate matmuls per class:
    E = sum_j X_j X_j^T. The -N m m^T correction happens on the host.
  host:     cova = E/(N-1) - N/(N-1) m m^T, cholesky for the ACT classes.
  launch 2: shard the 256 queries over 8 cores (32 each), processed in
    groups of 8 so each class's stationary is loaded once per 8 queries.
    Even classes: U = L_w^T qn (PE bf16), V = U^2 (ACT square).
    Odd classes:  U = C_w qn  (PE bf16), V = U * qn (DVE mult).
    Projections run in 4-query bursts alternating even/odd classes so ACT
    and DVE evacuate PSUM concurrently while stationaries amortize over
    4 queries each.
    sim rows = ones^T V (PE ones-matmuls, 4 classes packed per psum tile
    via col tile_position), staged to SBUF (ACT/DVE alternate), DMA out.
    The channel-norm square+accumulate runs on the Pool engine.
"""

import os

import numpy as np

import concourse.bass as bass
import concourse.tile as tile
from concourse import bacc, mybir
from concourse.bass_utils import run_bass_kernel_spmd

F32 = mybir.dt.float32
BF16 = mybir.dt.bfloat16

N_CORES = 8
B, C, HW = 256, 128, 1024          # x1: [B, C, 32, 32]
W, S = 16, 8                       # x2: [W, S, C, 32, 32]
NS = S * HW                        # samples per class = 8192
NCHUNK = NS // 128                 # 64 pixel-chunks per class
BS = B // N_CORES                  # 32 queries per core
WS = W // N_CORES                  # 2 classes per core
GB = 8                             # query group size (stationary reuse)

_CACHE = {}


def _build_cova_nc():
    """Per-core: x2t pair [2, 128, 64, 128] bf16 -> E pair [2, 128, 128] f32
    (raw sum_j X_j X_j^T; scaling and mean correction happen on the host)."""
    nc = bacc.Bacc("TRN2", target_bir_lowering=False, debug=False,
                   num_devices=N_CORES)
    x2t = nc.dram_tensor("x2t", [WS, 128, NCHUNK, C], BF16,
                         kind="ExternalInput").ap()
    eout = nc.dram_tensor("e_pair", [WS, C, C], F32, kind="ExternalOutput").ap()

    with tile.TileContext(nc) as tc:
        with (
            tc.tile_pool(name="xt", bufs=2) as xtp,
            tc.tile_pool(name="eo", bufs=2) as eop,
            tc.tile_pool(name="pe", bufs=2, space="PSUM") as pep,
        ):
            for w in range(WS):
                xt = xtp.tile([128, NCHUNK, C], BF16, tag="xt")
                nc.default_dma_engine.dma_start(out=xt, in_=x2t[w])
                e_ps = pep.tile([C, C], F32, tag="E")
                for j in range(NCHUNK):
                    xj = xt[:, j, :]
                    nc.tensor.matmul(e_ps, xj, xj,
                                     start=(j == 0), stop=(j == NCHUNK - 1))
                esb = eop.tile([C, C], F32, tag="esb")
                if w % 2 == 0:
                    nc.scalar.copy(esb, e_ps)
                else:
                    nc.vector.tensor_copy(esb, e_ps)
                nc.default_dma_engine.dma_start(out=eout[w], in_=esb)

    nc.compile()
    return nc


def _build_sim_nc():
    """Per-core: x1 shard [32, 128, 1024] bf16 + mats [16, 128, 128] ->
    sim shard [32, 16 * 1024] f32.

    mats[w] = chol(cova[w]) for even w else cova[w], prepacked on host.
    """
    nc = bacc.Bacc("TRN2", target_bir_lowering=False, debug=False,
                   num_devices=N_CORES)
    x1s = nc.dram_tensor("x1s", [BS, C, HW], BF16, kind="ExternalInput").ap()
    mats = nc.dram_tensor("mats", [W, C, C], F32, kind="ExternalInput").ap()
    out = nc.dram_tensor("sim", [BS, W, HW], F32, kind="ExternalOutput").ap()

    HC = 512  # matmul moving-dim chunk

    with tile.TileContext(nc) as tc:
        with (
            tc.tile_pool(name="consts", bufs=1) as consts,
            tc.tile_pool(name="mats", bufs=1) as matp,
            tc.tile_pool(name="xb", bufs=GB + 2) as xbp,
            tc.tile_pool(name="sq", bufs=2) as sqp,
            tc.tile_pool(name="qn", bufs=GB + 1) as qnp,
            tc.tile_pool(name="vv", bufs=4 * GB + 2) as vvp,
            tc.tile_pool(name="st", bufs=4) as stp,
            tc.tile_pool(name="small", bufs=2 * GB + 2) as small,
            tc.tile_pool(name="pu", bufs=3, space="PSUM") as pu,
            tc.tile_pool(name="pr", bufs=2, space="PSUM") as pr,
        ):
            # M=32 all-ones stationary: every column of a col-group strip
            # computes the same k-sum; row j*32 of the strip carries class j's
            # sums.
            ones_r = consts.tile([C, 32], BF16)
            nc.vector.memset(ones_r, 1.0)

            # stationary matrices in bf16 (1 cycle/row at any p-state)
            mat_raw = matp.tile([C, W, C], F32)
            for w in range(W):
                nc.default_dma_engine.dma_start(out=mat_raw[:, w, :], in_=mats[w])
            mat = matp.tile([C, W, C], BF16)
            nc.vector.tensor_copy(mat, mat_raw)

            for bg in range(BS // GB):
                bs = list(range(bg * GB, (bg + 1) * GB))
                qns = {}
                for b in bs:
                    xb = xbp.tile([C, HW], BF16, tag="xb")
                    nc.default_dma_engine.dma_start(out=xb, in_=x1s[b])
                    # channel norms: n2 = sum_i x^2
                    sq = sqp.tile([C, HW], BF16, tag="sq")
                    n2 = small.tile([C, 1], F32, tag="n2")
                    nc.scalar.activation(sq, xb,
                                         mybir.ActivationFunctionType.Square,
                                         accum_out=n2)
                    nrm = small.tile([C, 1], F32, tag="nrm")
                    nc.scalar.sqrt(nrm, n2)
                    rinv = small.tile([C, 1], F32, tag="rinv")
                    nc.vector.reciprocal(rinv, nrm)
                    # all-bf16 SBUF operands -> DVE 4x mode
                    qn = qnp.tile([C, HW], BF16, tag="qn")
                    nc.vector.tensor_scalar_mul(qn, xb, rinv)
                    qns[b] = qn

                for g in range(4):
                    # projections class-major: one stationary load serves all
                    # GB queries (2 matmuls each); V evacuation trails on
                    # ACT/DVE.
                    vs = {}
                    for j2 in range(2):
                        for rep in range(2):
                            for parity in range(2):
                                j = 2 * j2 + parity
                                w = 4 * g + j
                                lw = mat[:, w, :]
                                for b in bs[4 * rep : 4 * rep + 4]:
                                    u_ps = pu.tile([C, HW], F32, tag="u")
                                    for h in range(2):
                                        cols = slice(h * HC, (h + 1) * HC)
                                        nc.tensor.matmul(
                                            u_ps[:, cols], lw,
                                            qns[b][:, cols],
                                            start=True, stop=True)
                                    v = vvp.tile([C, HW], BF16, tag="v")
                                    if w % 2 == 0:
                                        nc.scalar.square(v, u_ps)
                                    else:
                                        nc.vector.tensor_mul(v, u_ps, qns[b])
                                    vs[(b, j)] = v
                    # reduces per query: j-major, h pairs share the ones
                    # stationary position so walrus elides the reload.
                    for b in bs:
                        red0 = pr.tile([C, HC], F32, tag="red")
                        red1 = pr.tile([C, HC], F32, tag="red")
                        reds = [red0, red1]
                        for j in range(4):
                            for h in range(2):
                                cols = slice(h * HC, (h + 1) * HC)
                                nc.tensor.matmul(
                                    reds[h][32 * j : 32 * j + 32, :],
                                    ones_r, vs[(b, j)][:, cols],
                                    start=True, stop=True,
                                    tile_position=(0, 32 * j))
                        for h in range(2):
                            stage = stp.tile([C, HC], F32, tag="stage")
                            if (b + h) % 2 == 0:
                                nc.scalar.copy(stage, reds[h])
                            else:
                                nc.vector.tensor_copy(stage, reds[h])
                            srows = stage.rearrange(
                                "(j p) n -> j p n", p=32)[:, 0, :]
                            nc.default_dma_engine.dma_start(
                                out=out[b, 4 * g : 4 * g + 4,
                                        h * HC : (h + 1) * HC],
                                in_=srows)

    nc.compile()
    return nc


def kernel(x1: np.ndarray, x2: np.ndarray) -> np.ndarray:
    import ml_dtypes
    x1 = np.asarray(x1, dtype=np.float32).reshape(B, C, HW)
    x2 = np.asarray(x2, dtype=np.float32).reshape(W, S, C, HW)
    core_ids = list(range(N_CORES))

    profile = bool(os.environ.get("COVA_PROFILE"))
    kw1, kw2 = {}, {}
    if profile:
        import shutil
        for d in ("/tmp/cova_prof1", "/tmp/cova_prof2"):
            shutil.rmtree(d, ignore_errors=True)
            os.makedirs(d)
        kw1 = dict(trace=True, tmpdir="/tmp/cova_prof1")
        kw2 = dict(trace=True, tmpdir="/tmp/cova_prof2")

    # host: transpose supports to [W, 128 pix, 64 chunk, 128 ch] bf16
    x2t = np.ascontiguousarray(
        x2.reshape(W, S, C, 8, 128).transpose(0, 4, 1, 3, 2)
        .reshape(W, 128, NCHUNK, C).astype(ml_dtypes.bfloat16))

    if "cova" not in _CACHE:
        _CACHE["cova"] = _build_cova_nc()
    cova_in = [{"x2t": np.ascontiguousarray(x2t[WS * k : WS * (k + 1)])}
               for k in range(N_CORES)]
    res1 = run_bass_kernel_spmd(_CACHE["cova"], cova_in, core_ids, **kw1)
    E = np.concatenate([res1.results[k]["e_pair"] for k in range(N_CORES)], 0)

    # host: cova = E/(N-1) - N/(N-1) m m^T  (mean from the f32 data)
    m = x2.transpose(0, 2, 1, 3).reshape(W, C, NS).mean(axis=2)  # [W, C]
    cova = E.astype(np.float64) / (NS - 1) - (
        NS / (NS - 1.0)) * np.einsum("wc,wd->wcd", m, m, dtype=np.float64)
    chol = np.linalg.cholesky(cova).astype(np.float32)
    mats = np.where((np.arange(W) % 2 == 0)[:, None, None],
                    chol, cova.astype(np.float32))
    mats = np.ascontiguousarray(mats.astype(np.float32))

    x1b = np.ascontiguousarray(x1.astype(ml_dtypes.bfloat16))
    if "sim" not in _CACHE:
        _CACHE["sim"] = _build_sim_nc()
    sim_in = [{"x1s": np.ascontiguousarray(x1b[BS * k : BS * (k + 1)]),
               "mats": mats} for k in range(N_CORES)]
    res2 = run_bass_kernel_spmd(_CACHE["sim"], sim_in, core_ids, **kw2)
    if profile:
        _CACHE["exec_ns"] = (res1.exec_time_ns, res2.exec_time_ns)
    sim = np.concatenate([res2.results[k]["sim"] for k in range(N_CORES)], 0)
    return sim.reshape(B, W * HW)
